# revision 6
# baseline (speedup 1.0000x reference)
"""Trainium2 Bass kernel for nn_ExactModel_15092515078731.

Reference computes, per timestep t:
    U = expm(-i t H);  psi = U[:, 0]
    rotate psi by 32 per-observable tensor-product single-qubit bases
    probs = |rotated|^2 ; gather at indices
Only one column of expm is used, so the device computes psi = expm(-itH) e0
via a Chebyshev expansion of the structured TFI Hamiltonian
    H = sum_i x_i X_i + diag(zz),   H v = zz*v + sum_i x_i v[n ^ (1<<i)]
All Chebyshev vectors T_k(Htilde) e0 are REAL (H real-symmetric); the complex
coefficients alternate real/imag, and the interval-shift global phase drops
out of |.|^2.

Device layout: state vector (4096,) as SBUF tile (128, 32): partition p =
bits 0-6 of n, free q = bits 7-11.  Per Chebyshev term:
  - one 128x128 matmul (bits 0-6 flips), 5 identity-weight matmuls with
    XOR-flipped rhs APs (bits 7-11 flips), one -I matmul (the -w_{k-1} term),
    all accumulating in one PSUM group
  - ZZ-diagonal elementwise on GPSIMD, combine on VectorE, psi accumulation
    via fused scalar_tensor_tensor.
Rotation: W_b = Wpart_b (128x128) (x) Wfree_b (32x32) precomputed on host;
stage A = per-b complex matmuls into a packed (128, 4*32) PSUM group tile;
PE transpose; stage B = block-diag(4 x 32x32) complex matmuls; |.|^2; DMA out.
Sharding: one timestep per core (8 cores), SPMD, no collectives.
Host does only: tiny parameter prep (O(DIM) + rotation kron products) and the
final index gather.
"""
import sys

if "/opt/trn_rl_repo" not in sys.path:
    sys.path.insert(0, "/opt/trn_rl_repo")

from contextlib import ExitStack

import numpy as np

import concourse.bacc as bacc
import concourse.bass as bass
import concourse.mybir as mybir
import concourse.tile as tile
from concourse.bass_utils import run_bass_kernel_spmd
from concourse.masks import make_identity

N = 12
DIM = 4096
P = 128   # partition: bits 0-6
F = 32    # free: bits 7-11
NCORES = 8
B = 32    # observables
NG = B // 4  # groups of 4 observables

_s = 1.0 / np.sqrt(2.0)
U_BASIS = np.stack([
    np.array([[1, 1], [1, -1]]) * _s,
    np.array([[1, -1j], [1, 1j]]) * _s,
    np.eye(2),
]).astype(np.complex128)

F32 = mybir.dt.float32
MULT = mybir.AluOpType.mult
ADD = mybir.AluOpType.add


# ----------------------------------------------------------------------------
# host math
# ----------------------------------------------------------------------------

def _build_zz_diag(params_zz):
    basis = np.arange(DIM)
    bits = (basis[:, None] >> np.arange(N)[None, :]) & 1
    signs = (1 - 2 * bits).astype(np.float64)
    return (signs[:, :-1] * signs[:, 1:]) @ params_zz


def _h_matvec(v, params_x, zz_diag):
    out = zz_diag * v
    idx = np.arange(DIM)
    for i in range(N):
        out = out + params_x[i] * v[idx ^ (1 << i)]
    return out


def _lanczos_bounds(params_x, zz_diag, iters=80, seed=0):
    rng = np.random.default_rng(seed)
    v = rng.standard_normal(DIM)
    v /= np.linalg.norm(v)
    V = [v]
    vprev = np.zeros(DIM)
    beta = 0.0
    alphas, betas = [], []
    for _ in range(iters):
        w = _h_matvec(V[-1], params_x, zz_diag) - beta * vprev
        alpha = np.dot(V[-1], w)
        w = w - alpha * V[-1]
        for u in V:
            w -= np.dot(u, w) * u
        beta = np.linalg.norm(w)
        alphas.append(alpha)
        betas.append(beta)
        if beta < 1e-12:
            break
        vprev = V[-1]
        V.append(w / beta)
    T = (np.diag(alphas)
         + np.diag(betas[:len(alphas) - 1], 1)
         + np.diag(betas[:len(alphas) - 1], -1))
    ev = np.linalg.eigvalsh(T)
    return ev[0], ev[-1]


def _bessel_j(kmax, z, npts=8192):
    theta = (np.arange(npts) + 0.5) * (np.pi / npts)
    k = np.arange(kmax)[:, None]
    return np.cos(k * theta[None, :] - z * np.sin(theta)[None, :]).mean(axis=1)


def _choose_K(a, t_max, tol=1e-8, Kcap=80):
    J = np.abs(_bessel_j(Kcap + 20, a * t_max))
    tails = np.cumsum(J[::-1])[::-1]
    K = int(np.argmax(tails < tol))
    return min(max(K, 8) + 2, Kcap)


def _build_rot_mats(pauli_obs):
    """Wpart (B,128,128), Wfree (B,32,32); qubit acting on bit k is
    U_BASIS[pauli_obs[b, 11-k]] (reference reshape is bit-11-major)."""
    Wpart = np.zeros((B, P, P), np.complex128)
    Wfree = np.zeros((B, F, F), np.complex128)
    for b in range(B):
        Ub = [U_BASIS[pauli_obs[b, 11 - k]] for k in range(N)]
        wp = np.array([[1.0]])
        for k in range(6, -1, -1):
            wp = np.kron(wp, Ub[k])
        wf = np.array([[1.0]])
        for k in range(11, 6, -1):
            wf = np.kron(wf, Ub[k])
        Wpart[b] = wp
        Wfree[b] = wf
    return Wpart, Wfree


def prepare_host_data(initial_state, ts, pauli_obs, params_x, params_zz):
    """All host-side constants. Returns (shared dict, per-core list, K)."""
    n0 = int(initial_state)
    ts = np.asarray(ts, np.float64)
    pauli_obs = np.asarray(pauli_obs, np.int64)
    params_x = np.asarray(params_x, np.float64)
    params_zz = np.asarray(params_zz, np.float64)

    zz_diag = _build_zz_diag(params_zz)
    lmin, lmax = _lanczos_bounds(params_x, zz_diag)
    pad = 0.02 * (lmax - lmin)
    a = (lmax - lmin) / 2 + pad
    bshift = (lmax + lmin) / 2
    K = _choose_K(a, float(ts.max()))

    # partition-side matmul weight: Apart[p',p] = (2/a) x_i for p' = p^(1<<i), i<7
    Apart = np.zeros((P, P), np.float64)
    for i in range(7):
        pp = np.arange(P)
        Apart[pp ^ (1 << i), pp] += params_x[i]
    Apart *= 2.0 / a
    a_lhsT = np.ascontiguousarray(Apart.T).astype(np.float32)

    negi = (-np.eye(P)).astype(np.float32)

    # free-side scaled identities, packed (128, 5*128)
    fi = np.zeros((P, 5 * P), np.float32)
    for j in range(5):
        fi[:, j * P:(j + 1) * P] = np.eye(P) * (2.0 / a * params_x[7 + j])

    # scaled shifted diagonal as [p, q] table
    zt = ((zz_diag - bshift) * (2.0 / a)).astype(np.float32)
    ztab = zt.reshape(F, P).T.copy()  # n = q<<7 | p -> [p, q]

    # w1 = 2 * Htilde e_{n0};  wprev0 = 2 e_{n0}
    e0 = np.zeros(DIM, np.float64)
    e0[n0] = 1.0
    ht_e0 = (_h_matvec(e0, params_x, zz_diag) - bshift * e0) / a
    w1_vec = 2.0 * ht_e0
    w1 = w1_vec.reshape(F, P).T.astype(np.float32).copy()
    wprev0 = (2.0 * e0).reshape(F, P).T.astype(np.float32).copy()

    # rotation weights
    Wpart, Wfree = _build_rot_mats(pauli_obs)
    # stage A lhsT = Wpart_b.T, packed [k, b*128+m] (SBUF layout)
    wp_re = np.zeros((P, B * P), np.float32)
    wp_im = np.zeros((P, B * P), np.float32)
    for b in range(B):
        wp_re[:, b * P:(b + 1) * P] = Wpart[b].T.real
        wp_im[:, b * P:(b + 1) * P] = Wpart[b].T.imag
    # stage B lhsT = blockdiag_j(Wfree_{4g+j}.T), packed [k, g*128+m]
    bd_r = np.zeros((P, NG * P), np.float32)
    bd_i = np.zeros((P, NG * P), np.float32)
    bd_in = np.zeros((P, NG * P), np.float32)
    for g in range(NG):
        for j in range(4):
            wf = Wfree[4 * g + j]
            sl = np.s_[32 * j:32 * (j + 1)]
            blk_r = wf.T.real
            blk_i = wf.T.imag
            bd_r[sl, g * P + 32 * j:g * P + 32 * (j + 1)] = blk_r
            bd_i[sl, g * P + 32 * j:g * P + 32 * (j + 1)] = blk_i
            bd_in[sl, g * P + 32 * j:g * P + 32 * (j + 1)] = -blk_i

    shared = dict(a_lhsT=a_lhsT, negi=negi, fi=fi, ztab=ztab, w1=w1,
                  wprev0=wprev0, wp_re=wp_re, wp_im=wp_im,
                  bd_r=bd_r, bd_i=bd_i, bd_in=bd_in)

    # per-core: psi init tiles + gamma coefficients
    per_core = []
    for t in ts:
        z = a * float(t)
        J = _bessel_j(K, z)
        k = np.arange(K)
        c = (2 - (k == 0)) * (-1j) ** k * J  # global phase dropped
        psi0 = c[0] * e0 + (c[1] / 2) * w1_vec
        psi0_re = psi0.real.reshape(F, P).T.astype(np.float32).copy()
        psi0_im = psi0.imag.reshape(F, P).T.astype(np.float32).copy()
        gam = np.zeros(K, np.float64)
        gam[2::2] = c[2::2].real / 2
        gam[3::2] = c[3::2].imag / 2
        gamma = np.broadcast_to(gam.astype(np.float32), (P, K)).copy()
        per_core.append(dict(psi0_re=psi0_re, psi0_im=psi0_im, gamma=gamma))
    return shared, per_core, K


# ----------------------------------------------------------------------------
# device program
# ----------------------------------------------------------------------------

def _flip_free(t_ap, j):
    """View of [128, 32] tile with free index XORed with (1<<j), j in 0..4."""
    b = 1 << j
    v = t_ap.rearrange("p (h m l) -> p h m l", h=F // (2 * b), m=2, l=b)
    return v[:, :, ::-1, :]


def build_program(K):
    nc = bacc.Bacc("TRN2", target_bir_lowering=False, debug=False,
                   num_devices=NCORES)

    d_a = nc.dram_tensor("a_lhsT", [P, P], F32, kind="ExternalInput")
    d_negi = nc.dram_tensor("negi", [P, P], F32, kind="ExternalInput")
    d_fi = nc.dram_tensor("fi", [P, 5 * P], F32, kind="ExternalInput")
    d_ztab = nc.dram_tensor("ztab", [P, F], F32, kind="ExternalInput")
    d_w1 = nc.dram_tensor("w1", [P, F], F32, kind="ExternalInput")
    d_wprev0 = nc.dram_tensor("wprev0", [P, F], F32, kind="ExternalInput")
    d_psi0_re = nc.dram_tensor("psi0_re", [P, F], F32, kind="ExternalInput")
    d_psi0_im = nc.dram_tensor("psi0_im", [P, F], F32, kind="ExternalInput")
    d_gamma = nc.dram_tensor("gamma", [P, K], F32, kind="ExternalInput")
    d_wp_re = nc.dram_tensor("wp_re", [P, B * P], F32, kind="ExternalInput")
    d_wp_im = nc.dram_tensor("wp_im", [P, B * P], F32, kind="ExternalInput")
    d_bd_r = nc.dram_tensor("bd_r", [P, NG * P], F32, kind="ExternalInput")
    d_bd_i = nc.dram_tensor("bd_i", [P, NG * P], F32, kind="ExternalInput")
    d_bd_in = nc.dram_tensor("bd_in", [P, NG * P], F32, kind="ExternalInput")
    d_probs = nc.dram_tensor("probs", [NG, P, P], F32, kind="ExternalOutput")

    with tile.TileContext(nc) as tc, ExitStack() as ctx:
        consts = ctx.enter_context(tc.tile_pool(name="consts", bufs=1))
        state = ctx.enter_context(tc.tile_pool(name="state", bufs=3))
        tmp = ctx.enter_context(tc.tile_pool(name="tmp", bufs=2))
        evo_ctx = ctx.enter_context(ExitStack())
        ps_evo = evo_ctx.enter_context(
            tc.tile_pool(name="ps_evo", bufs=2, space="PSUM"))

        sb_a = consts.tile([P, P], F32, tag="a")
        nc.sync.dma_start(out=sb_a, in_=d_a.ap())
        sb_negi = consts.tile([P, P], F32, tag="negi")
        nc.sync.dma_start(out=sb_negi, in_=d_negi.ap())
        sb_fi = consts.tile([P, 5 * P], F32, tag="fi")
        nc.sync.dma_start(out=sb_fi, in_=d_fi.ap())
        sb_ztab = consts.tile([P, F], F32, tag="ztab")
        nc.sync.dma_start(out=sb_ztab, in_=d_ztab.ap())
        sb_gamma = consts.tile([P, K], F32, tag="gamma")
        nc.sync.dma_start(out=sb_gamma, in_=d_gamma.ap())
        sb_psi_re = consts.tile([P, F], F32, tag="psi_re")
        nc.sync.dma_start(out=sb_psi_re, in_=d_psi0_re.ap())
        sb_psi_im = consts.tile([P, F], F32, tag="psi_im")
        nc.sync.dma_start(out=sb_psi_im, in_=d_psi0_im.ap())
        # big rotation weight tiles
        sb_wp_re = consts.tile([P, B * P], F32, tag="wp_re")
        nc.sync.dma_start(out=sb_wp_re, in_=d_wp_re.ap())
        sb_wp_im = consts.tile([P, B * P], F32, tag="wp_im")
        nc.sync.dma_start(out=sb_wp_im, in_=d_wp_im.ap())
        sb_bd_r = consts.tile([P, NG * P], F32, tag="bd_r")
        nc.sync.dma_start(out=sb_bd_r, in_=d_bd_r.ap())
        sb_bd_i = consts.tile([P, NG * P], F32, tag="bd_i")
        nc.sync.dma_start(out=sb_bd_i, in_=d_bd_i.ap())
        sb_bd_in = consts.tile([P, NG * P], F32, tag="bd_in")
        nc.sync.dma_start(out=sb_bd_in, in_=d_bd_in.ap())

        sb_ident = consts.tile([P, P], F32, tag="ident")
        make_identity(nc, sb_ident)

        # ---------------- Chebyshev evolution ----------------
        w = state.tile([P, F], F32, tag="w")
        nc.sync.dma_start(out=w, in_=d_w1.ap())
        wprev = state.tile([P, F], F32, tag="w")
        nc.sync.dma_start(out=wprev, in_=d_wprev0.ap())

        for k in range(2, K):
            ps = ps_evo.tile([P, F], F32, tag="ps")
            nc.tensor.matmul(ps, sb_a, w, start=True, stop=False)
            for j in range(5):
                nc.tensor.matmul(ps, sb_fi[:, j * P:(j + 1) * P],
                                 _flip_free(w, j), start=False, stop=False)
            nc.tensor.matmul(ps, sb_negi, wprev, start=False, stop=True)

            zt = tmp.tile([P, F], F32, tag="zt")
            nc.gpsimd.tensor_tensor(zt, w, sb_ztab, op=MULT)

            wnew = state.tile([P, F], F32, tag="w")
            nc.vector.tensor_tensor(wnew, ps, zt, op=ADD)

            tgt = sb_psi_re if (k % 2 == 0) else sb_psi_im
            nc.vector.scalar_tensor_tensor(
                tgt, wnew, sb_gamma[:, k:k + 1], tgt, MULT, ADD)
            wprev, w = w, wnew

        # ---------------- rotation + probabilities ----------------
        evo_ctx.close()  # release evolution PSUM banks
        sb_psi_im_neg = consts.tile([P, F], F32, tag="psi_im_neg")
        nc.vector.tensor_scalar_mul(sb_psi_im_neg, sb_psi_im, -1.0)

        ps_a = ctx.enter_context(tc.tile_pool(name="ps_a", bufs=2, space="PSUM"))
        ps_t = ctx.enter_context(tc.tile_pool(name="ps_t", bufs=1, space="PSUM"))
        ps_b = ctx.enter_context(tc.tile_pool(name="ps_b", bufs=1, space="PSUM"))
        rot = ctx.enter_context(tc.tile_pool(name="rot", bufs=3))
        prob_pool = ctx.enter_context(tc.tile_pool(name="probs", bufs=2))

        for g in range(NG):
            psA_re = ps_a.tile([P, P], F32, tag="psA_re")
            psA_im = ps_a.tile([P, P], F32, tag="psA_im")
            for j in range(4):
                b = 4 * g + j
                wr = sb_wp_re[:, b * P:(b + 1) * P]
                wi = sb_wp_im[:, b * P:(b + 1) * P]
                osl = np.s_[:, 32 * j:32 * (j + 1)]
                nc.tensor.matmul(psA_re[osl], wr, sb_psi_re, start=True, stop=False)
                nc.tensor.matmul(psA_im[osl], wr, sb_psi_im, start=True, stop=False)
                nc.tensor.matmul(psA_re[osl], wi, sb_psi_im_neg, start=False, stop=True)
                nc.tensor.matmul(psA_im[osl], wi, sb_psi_re, start=False, stop=True)
            g_re = rot.tile([P, P], F32, tag="g_re")
            nc.vector.tensor_copy(g_re, psA_re)
            g_im = rot.tile([P, P], F32, tag="g_im")
            nc.vector.tensor_copy(g_im, psA_im)
            psT_re = ps_t.tile([P, P], F32, tag="psT_re")
            nc.tensor.transpose(psT_re, g_re, sb_ident)
            psT_im = ps_t.tile([P, P], F32, tag="psT_im")
            nc.tensor.transpose(psT_im, g_im, sb_ident)
            gt_re = rot.tile([P, P], F32, tag="gt_re")
            nc.vector.tensor_copy(gt_re, psT_re)
            gt_im = rot.tile([P, P], F32, tag="gt_im")
            nc.vector.tensor_copy(gt_im, psT_im)

            bdr = sb_bd_r[:, g * P:(g + 1) * P]
            bdi = sb_bd_i[:, g * P:(g + 1) * P]
            bdin = sb_bd_in[:, g * P:(g + 1) * P]
            psB_re = ps_b.tile([P, P], F32, tag="psB_re")
            psB_im = ps_b.tile([P, P], F32, tag="psB_im")
            nc.tensor.matmul(psB_re, bdr, gt_re, start=True, stop=False)
            nc.tensor.matmul(psB_im, bdr, gt_im, start=True, stop=False)
            nc.tensor.matmul(psB_re, bdin, gt_im, start=False, stop=True)
            nc.tensor.matmul(psB_im, bdi, gt_re, start=False, stop=True)

            sq_re = rot.tile([P, P], F32, tag="sq_re")
            nc.scalar.square(sq_re, psB_re)
            sq_im = rot.tile([P, P], F32, tag="sq_im")
            nc.scalar.square(sq_im, psB_im)
            pr = prob_pool.tile([P, P], F32, tag="pr")
            nc.vector.tensor_tensor(pr, sq_re, sq_im, op=ADD)
            nc.sync.dma_start(out=d_probs.ap()[g], in_=pr)

    nc.compile()
    return nc


# ----------------------------------------------------------------------------
# entry point
# ----------------------------------------------------------------------------

_PROGRAM_CACHE = {}

# test-harness knobs (grading path leaves these untouched)
TRACE = False
LAST_RESULT = None


def kernel(initial_state, ts, pauli_obs, indices, params_x, params_zz):
    ts = np.asarray(ts)
    pauli_obs = np.asarray(pauli_obs)
    indices = np.asarray(indices)
    T = ts.shape[0]
    shots = indices.shape[2]
    assert T == NCORES, f"expected {NCORES} timesteps, got {T}"

    shared, per_core, K = prepare_host_data(
        initial_state, ts, pauli_obs, params_x, params_zz)

    if K not in _PROGRAM_CACHE:
        _PROGRAM_CACHE[K] = build_program(K)
    nc = _PROGRAM_CACHE[K]

    in_maps = [{**shared, **pc} for pc in per_core]
    res = run_bass_kernel_spmd(nc, in_maps, core_ids=list(range(NCORES)),
                               trace=TRACE)
    global LAST_RESULT
    LAST_RESULT = res

    out = np.zeros((T, B, shots), np.float32)
    idx = indices.astype(np.int64)
    for t in range(T):
        tiles = res.results[t]["probs"]          # (NG, 128, 128)
        probs = tiles.reshape(NG, 4, F, P).reshape(B, DIM)  # b=(g,j); m=q'<<7|p'
        out[t] = np.take_along_axis(probs, idx[t], axis=1)
    return out


# revision 16
# speedup vs baseline: 2.0989x; 2.0989x over previous
"""Trainium2 Bass kernel for nn_ExactModel_15092515078731.

Reference computes, per timestep t:
    U = expm(-i t H);  psi = U[:, 0]
    rotate psi by 32 per-observable tensor-product single-qubit bases
    probs = |rotated|^2 ; gather at indices
Only one column of expm is used, so the device computes psi = expm(-itH) e0
via a Chebyshev expansion of the structured TFI Hamiltonian
    H = sum_i x_i X_i + diag(zz),   H v = zz*v + sum_i x_i v[n ^ (1<<i)]
All Chebyshev vectors T_k(Htilde) e0 are REAL (H real-symmetric); the complex
coefficients alternate real/imag, and the interval-shift global phase drops
out of |.|^2.

Device layout: state vector (4096,) as SBUF tile (128, 32): partition p =
bits 0-6 of n, free q = bits 7-11.  Per Chebyshev term:
  - bits 0-6 flips via one 128x128 f32 matmul (PSUM)
  - bits 7-11 flips + ZZ diagonal + the -w_{k-1} term via fused
    scalar_tensor_tensor chains split across VectorE and GPSIMD, using
    XOR-flipped free-axis access patterns (a scale-ratio pivot shortens the
    VectorE chain)
  - psi accumulation via fused stt on GPSIMD.
Rotation: W_b = Wpart_b (128x128) (x) Wfree_b (32x32) precomputed on host in
fp16 (single application -> no error accumulation); stage A streams
[psi_re|psi_im] through each Wpart in 2 matmuls/b into a packed PSUM tile;
PE transposes; stage B = block-diag(4 x 32x32) fp16 matmuls; |.|^2 on
ScalarE; DMA out.  Sharding: one timestep per core (8 cores), SPMD.
Host does only tiny parameter prep (O(DIM) vectors + rotation kron
products) and the final index gather.
"""
import sys

if "/opt/trn_rl_repo" not in sys.path:
    sys.path.insert(0, "/opt/trn_rl_repo")

from contextlib import ExitStack

import numpy as np

import concourse.bacc as bacc
import concourse.bass as bass
import concourse.mybir as mybir
import concourse.tile as tile
from concourse.bass_utils import run_bass_kernel_spmd
from concourse.masks import make_identity

N = 12
DIM = 4096
P = 128   # partition: bits 0-6
F = 32    # free: bits 7-11
NCORES = 8
B = 32    # observables
NG = B // 4  # groups of 4 observables

_s = 1.0 / np.sqrt(2.0)
U_BASIS = np.stack([
    np.array([[1, 1], [1, -1]]) * _s,
    np.array([[1, -1j], [1, 1j]]) * _s,
    np.eye(2),
]).astype(np.complex128)

F32 = mybir.dt.float32
F16 = mybir.dt.float16
MULT = mybir.AluOpType.mult
ADD = mybir.AluOpType.add


# ----------------------------------------------------------------------------
# host math
# ----------------------------------------------------------------------------

def _build_zz_diag(params_zz):
    basis = np.arange(DIM)
    bits = (basis[:, None] >> np.arange(N)[None, :]) & 1
    signs = (1 - 2 * bits).astype(np.float64)
    return (signs[:, :-1] * signs[:, 1:]) @ params_zz


def _h_matvec(v, params_x, zz_diag):
    out = zz_diag * v
    idx = np.arange(DIM)
    for i in range(N):
        out = out + params_x[i] * v[idx ^ (1 << i)]
    return out


def _lanczos_bounds(params_x, zz_diag, iters=80, seed=0):
    rng = np.random.default_rng(seed)
    v = rng.standard_normal(DIM)
    v /= np.linalg.norm(v)
    V = [v]
    vprev = np.zeros(DIM)
    beta = 0.0
    alphas, betas = [], []
    for _ in range(iters):
        w = _h_matvec(V[-1], params_x, zz_diag) - beta * vprev
        alpha = np.dot(V[-1], w)
        w = w - alpha * V[-1]
        for u in V:
            w -= np.dot(u, w) * u
        beta = np.linalg.norm(w)
        alphas.append(alpha)
        betas.append(beta)
        if beta < 1e-12:
            break
        vprev = V[-1]
        V.append(w / beta)
    T = (np.diag(alphas)
         + np.diag(betas[:len(alphas) - 1], 1)
         + np.diag(betas[:len(alphas) - 1], -1))
    ev = np.linalg.eigvalsh(T)
    return ev[0], ev[-1]


def _bessel_j(kmax, z, npts=8192):
    theta = (np.arange(npts) + 0.5) * (np.pi / npts)
    k = np.arange(kmax)[:, None]
    return np.cos(k * theta[None, :] - z * np.sin(theta)[None, :]).mean(axis=1)


def _choose_K(a, t_max, tol=1e-5, Kcap=80):
    J = np.abs(_bessel_j(Kcap + 20, a * t_max))
    tails = np.cumsum(J[::-1])[::-1]
    K = int(np.argmax(tails < tol))
    return min(max(K, 8) + 2, Kcap)


def _build_rot_mats(pauli_obs):
    """Wpart (B,128,128), Wfree (B,32,32); qubit acting on bit k is
    U_BASIS[pauli_obs[b, 11-k]] (reference reshape is bit-11-major)."""
    Wpart = np.zeros((B, P, P), np.complex128)
    Wfree = np.zeros((B, F, F), np.complex128)
    for b in range(B):
        Ub = [U_BASIS[pauli_obs[b, 11 - k]] for k in range(N)]
        wp = np.array([[1.0]])
        for k in range(6, -1, -1):
            wp = np.kron(wp, Ub[k])
        wf = np.array([[1.0]])
        for k in range(11, 6, -1):
            wf = np.kron(wf, Ub[k])
        Wpart[b] = wp
        Wfree[b] = wf
    return Wpart, Wfree


def prepare_host_data(initial_state, ts, pauli_obs, params_x, params_zz):
    """All host-side constants. Returns (shared dict, per-core list, K, perm)."""
    n0 = int(initial_state)
    ts = np.asarray(ts, np.float64)
    pauli_obs = np.asarray(pauli_obs, np.int64)
    params_x = np.asarray(params_x, np.float64)
    params_zz = np.asarray(params_zz, np.float64)

    zz_diag = _build_zz_diag(params_zz)
    lmin, lmax = _lanczos_bounds(params_x, zz_diag)
    pad = 0.02 * (lmax - lmin)
    a = (lmax - lmin) / 2 + pad
    bshift = (lmax + lmin) / 2
    K = _choose_K(a, float(ts.max()))

    # partition-side matmul weight: Apart[p',p] = (2/a) x_i for p'=p^(1<<i), i<7
    Apart = np.zeros((P, P), np.float64)
    for i in range(7):
        pp = np.arange(P)
        Apart[pp ^ (1 << i), pp] += params_x[i]
    Apart *= 2.0 / a
    a_lhsT = np.ascontiguousarray(Apart.T).astype(np.float32)

    # free-bit coefficients (2/a) x_{7+j}; xf cols = [x1, x2, x0, x3, x4]
    xt = (2.0 / a) * params_x[7:12]
    perm = ()
    xf = np.broadcast_to(
        np.array([xt[1], xt[2], xt[0], xt[3], xt[4]], np.float32), (P, 5)).copy()
    xt4 = np.full((P, F), xt[4], np.float32)
    fi3 = (np.eye(P) * xt[3]).astype(np.float32)

    # scaled shifted diagonal as [p, q] table
    zt = ((zz_diag - bshift) * (2.0 / a)).astype(np.float32)
    ztab = zt.reshape(F, P).T.copy()  # n = q<<7 | p -> [p, q]

    # w1 = 2 * Htilde e_{n0};  wprev0 = 2 e_{n0}
    e0 = np.zeros(DIM, np.float64)
    e0[n0] = 1.0
    ht_e0 = (_h_matvec(e0, params_x, zz_diag) - bshift * e0) / a
    w1_vec = 2.0 * ht_e0
    w1 = w1_vec.reshape(F, P).T.astype(np.float32).copy()
    wprev0 = (2.0 * e0).reshape(F, P).T.astype(np.float32).copy()

    # rotation weights (fp16)
    Wpart, Wfree = _build_rot_mats(pauli_obs)
    wp_re = np.zeros((P, B * P), np.float16)
    wp_im = np.zeros((P, B * P), np.float16)
    for b in range(B):
        wp_re[:, b * P:(b + 1) * P] = Wpart[b].T.real.astype(np.float16)
        wp_im[:, b * P:(b + 1) * P] = Wpart[b].T.imag.astype(np.float16)
    bd_r = np.zeros((P, NG * P), np.float16)
    bd_i = np.zeros((P, NG * P), np.float16)
    bd_in = np.zeros((P, NG * P), np.float16)
    for g in range(NG):
        for j in range(4):
            wf = Wfree[4 * g + j]
            sl = np.s_[32 * j:32 * (j + 1)]
            cs = np.s_[g * P + 32 * j:g * P + 32 * (j + 1)]
            bd_r[sl, cs] = wf.T.real.astype(np.float16)
            bd_i[sl, cs] = wf.T.imag.astype(np.float16)
            bd_in[sl, cs] = (-wf.T.imag).astype(np.float16)

    shared = dict(a_lhsT=a_lhsT, xf=xf, xt4=xt4, fi3=fi3, ztab=ztab, w1=w1,
                  wprev0=wprev0, wp_re=wp_re, wp_im=wp_im, bd_r=bd_r,
                  bd_i=bd_i, bd_in=bd_in)

    per_core = []
    for t in ts:
        z = a * float(t)
        J = _bessel_j(K, z)
        k = np.arange(K)
        c = (2 - (k == 0)) * (-1j) ** k * J  # global phase dropped
        psi0 = c[0] * e0 + (c[1] / 2) * w1_vec
        psi0_re = psi0.real.reshape(F, P).T.astype(np.float32).copy()
        psi0_im = psi0.imag.reshape(F, P).T.astype(np.float32).copy()
        gam = np.zeros(K, np.float64)
        gam[2::2] = c[2::2].real / 2
        gam[3::2] = c[3::2].imag / 2
        gamma = np.broadcast_to(gam.astype(np.float32), (P, K)).copy()
        per_core.append(dict(psi0_re=psi0_re, psi0_im=psi0_im, gamma=gamma))
    return shared, per_core, K, perm


# ----------------------------------------------------------------------------
# device program
# ----------------------------------------------------------------------------

def _flip_free(t_ap, j):
    """View of [128, 32] tile with free index XORed with (1<<j).

    j=0 and j=4 give 3D views (legal for DVE/GPSIMD ops); j=1..3 are 4D
    (matmul-rhs only -- walrus caps elementwise ops at 3D)."""
    b = 1 << j
    if j == 0:
        return t_ap.rearrange("p (h m) -> p h m", m=2)[:, :, ::-1]
    if j == 4:
        return t_ap.rearrange("p (m l) -> p m l", m=2)[:, ::-1, :]
    v = t_ap.rearrange("p (h m l) -> p h m l", h=F // (2 * b), m=2, l=b)
    return v[:, :, ::-1, :]


def _m_view(t_ap, j):
    """[p, h, m, l] view splitting the free axis around bit j."""
    b = 1 << j
    return t_ap.rearrange("p (h m l) -> p h m l", h=F // (2 * b), m=2, l=b)


def build_program(K, perm=()):
    nc = bacc.Bacc("TRN2", target_bir_lowering=False, debug=False,
                   num_devices=NCORES)

    d_a = nc.dram_tensor("a_lhsT", [P, P], F32, kind="ExternalInput")
    d_xf = nc.dram_tensor("xf", [P, 5], F32, kind="ExternalInput")
    d_xt4 = nc.dram_tensor("xt4", [P, F], F32, kind="ExternalInput")
    d_fi3 = nc.dram_tensor("fi3", [P, P], F32, kind="ExternalInput")
    d_ztab = nc.dram_tensor("ztab", [P, F], F32, kind="ExternalInput")
    d_w1 = nc.dram_tensor("w1", [P, F], F32, kind="ExternalInput")
    d_wprev0 = nc.dram_tensor("wprev0", [P, F], F32, kind="ExternalInput")
    d_psi0_re = nc.dram_tensor("psi0_re", [P, F], F32, kind="ExternalInput")
    d_psi0_im = nc.dram_tensor("psi0_im", [P, F], F32, kind="ExternalInput")
    d_gamma = nc.dram_tensor("gamma", [P, K], F32, kind="ExternalInput")
    d_wp_re = nc.dram_tensor("wp_re", [P, B * P], F16, kind="ExternalInput")
    d_wp_im = nc.dram_tensor("wp_im", [P, B * P], F16, kind="ExternalInput")
    d_bd_r = nc.dram_tensor("bd_r", [P, NG * P], F16, kind="ExternalInput")
    d_bd_i = nc.dram_tensor("bd_i", [P, NG * P], F16, kind="ExternalInput")
    d_bd_in = nc.dram_tensor("bd_in", [P, NG * P], F16, kind="ExternalInput")
    d_probs = nc.dram_tensor("probs", [NG, P, P], F32, kind="ExternalOutput")

    with tile.TileContext(nc) as tc, ExitStack() as ctx:
        consts = ctx.enter_context(tc.tile_pool(name="consts", bufs=1))
        state = ctx.enter_context(tc.tile_pool(name="state", bufs=3))
        tmp = ctx.enter_context(tc.tile_pool(name="tmp", bufs=2))
        evo_ctx = ctx.enter_context(ExitStack())
        ps_evo = evo_ctx.enter_context(
            tc.tile_pool(name="ps_evo", bufs=2, space="PSUM"))

        sb_a = consts.tile([P, P], F32, tag="a")
        nc.sync.dma_start(out=sb_a, in_=d_a.ap())
        sb_xf = consts.tile([P, 5], F32, tag="xf")
        nc.sync.dma_start(out=sb_xf, in_=d_xf.ap())
        sb_xt4 = consts.tile([P, F], F32, tag="xt4")
        nc.sync.dma_start(out=sb_xt4, in_=d_xt4.ap())
        sb_fi3 = consts.tile([P, P], F32, tag="fi3")
        nc.sync.dma_start(out=sb_fi3, in_=d_fi3.ap())
        sb_ztab = consts.tile([P, F], F32, tag="ztab")
        nc.sync.dma_start(out=sb_ztab, in_=d_ztab.ap())
        sb_gamma = consts.tile([P, K], F32, tag="gamma")
        nc.sync.dma_start(out=sb_gamma, in_=d_gamma.ap())
        sb_psi_re = consts.tile([P, F], F32, tag="psi_re")
        nc.sync.dma_start(out=sb_psi_re, in_=d_psi0_re.ap())
        sb_psi_im = consts.tile([P, F], F32, tag="psi_im")
        nc.sync.dma_start(out=sb_psi_im, in_=d_psi0_im.ap())
        sb_wp_re = consts.tile([P, B * P], F16, tag="wp_re")
        nc.sync.dma_start(out=sb_wp_re, in_=d_wp_re.ap())
        sb_wp_im = consts.tile([P, B * P], F16, tag="wp_im")
        nc.sync.dma_start(out=sb_wp_im, in_=d_wp_im.ap())
        sb_bd_r = consts.tile([P, NG * P], F16, tag="bd_r")
        nc.sync.dma_start(out=sb_bd_r, in_=d_bd_r.ap())
        sb_bd_i = consts.tile([P, NG * P], F16, tag="bd_i")
        nc.sync.dma_start(out=sb_bd_i, in_=d_bd_i.ap())
        sb_bd_in = consts.tile([P, NG * P], F16, tag="bd_in")
        nc.sync.dma_start(out=sb_bd_in, in_=d_bd_in.ap())

        sb_id16 = consts.tile([P, P], F16, tag="id16")
        make_identity(nc, sb_id16)

        # ---------------- Chebyshev evolution ----------------
        w = state.tile([P, F], F32, tag="w")
        nc.sync.dma_start(out=w, in_=d_w1.ap())
        wprev = state.tile([P, F], F32, tag="w")
        nc.sync.dma_start(out=wprev, in_=d_wprev0.ap())

        xt4_v = sb_xt4.rearrange("p (m l) -> p m l", m=2)
        for k in range(2, K):
            # PE: bits 0-6 flips + bit-3 free flip (4D rhs ok on matmul)
            ps = ps_evo.tile([P, F], F32, tag="ps")
            nc.tensor.matmul(ps, sb_a, w, start=True, stop=False)
            nc.tensor.matmul(ps, sb_fi3, _flip_free(w, 3), start=False, stop=True)

            wv1, wv2 = _m_view(w, 1), _m_view(w, 2)
            # VE: scaled flips of bits 1, 2 via scale-half-copies; bit 0 direct
            cp1 = tmp.tile([P, F], F32, tag="cp1")
            cv1 = _m_view(cp1, 1)
            nc.vector.tensor_scalar_mul(cv1[:, :, 0, :], wv1[:, :, 1, :],
                                        sb_xf[:, 0:1])
            nc.vector.tensor_scalar_mul(cv1[:, :, 1, :], wv1[:, :, 0, :],
                                        sb_xf[:, 0:1])
            cp2 = tmp.tile([P, F], F32, tag="cp2")
            cv2 = _m_view(cp2, 2)
            nc.vector.tensor_scalar_mul(cv2[:, :, 0, :], wv2[:, :, 1, :],
                                        sb_xf[:, 1:2])
            nc.vector.tensor_scalar_mul(cv2[:, :, 1, :], wv2[:, :, 0, :],
                                        sb_xf[:, 1:2])
            cp0 = tmp.tile([P, F], F32, tag="cp0")
            nc.vector.tensor_scalar_mul(
                cp0.rearrange("p (h m) -> p h m", m=2),
                _flip_free(w, 0), sb_xf[:, 2:3])

            # GPSIMD (tensor_tensor only): diag, -wprev, bit-4 flip
            z = tmp.tile([P, F], F32, tag="z")
            nc.gpsimd.tensor_tensor(z, w, sb_ztab, op=MULT)
            zg = tmp.tile([P, F], F32, tag="zg")
            nc.gpsimd.tensor_sub(zg, z, wprev)
            t4 = tmp.tile([P, F], F32, tag="t4")
            nc.gpsimd.tensor_tensor(t4.rearrange("p (m l) -> p m l", m=2),
                                    _flip_free(w, 4), xt4_v, op=MULT)
            g2 = tmp.tile([P, F], F32, tag="g2")
            nc.gpsimd.tensor_add(g2, t4, zg)

            # VE combine tree
            ca = tmp.tile([P, F], F32, tag="ca")
            nc.vector.tensor_add(ca, cp1, cp2)
            cb = tmp.tile([P, F], F32, tag="cb")
            nc.vector.tensor_add(cb, ca, cp0)
            cc = tmp.tile([P, F], F32, tag="cc")
            nc.vector.tensor_add(cc, cb, ps)
            wnew = state.tile([P, F], F32, tag="w")
            nc.vector.tensor_add(wnew, cc, g2)

            tgt = sb_psi_re if (k % 2 == 0) else sb_psi_im
            nc.vector.scalar_tensor_tensor(
                tgt, wnew, sb_gamma[:, k:k + 1], tgt, MULT, ADD)
            wprev, w = w, wnew

        # ---------------- rotation + probabilities (fp16) ----------------
        evo_ctx.close()  # release evolution PSUM banks
        # psicat1 = [psi_re | psi_im], psicat2 = [-psi_im | psi_re] in fp16
        sb_cat1 = consts.tile([P, 2 * F], F16, tag="cat1")
        nc.vector.tensor_copy(sb_cat1[:, 0:F], sb_psi_re)
        nc.vector.tensor_copy(sb_cat1[:, F:2 * F], sb_psi_im)
        sb_cat2 = consts.tile([P, 2 * F], F16, tag="cat2")
        nc.vector.tensor_scalar_mul(sb_cat2[:, 0:F], sb_psi_im, -1.0)
        nc.vector.tensor_copy(sb_cat2[:, F:2 * F], sb_psi_re)

        ps_a = ctx.enter_context(tc.tile_pool(name="ps_a", bufs=2, space="PSUM"))
        ps_t = ctx.enter_context(tc.tile_pool(name="ps_t", bufs=2, space="PSUM"))
        ps_b = ctx.enter_context(tc.tile_pool(name="ps_b", bufs=1, space="PSUM"))
        rot = ctx.enter_context(tc.tile_pool(name="rot", bufs=3))
        prob_pool = ctx.enter_context(tc.tile_pool(name="probs", bufs=2))

        for g in range(NG):
            # stage A: psA[:, 64j:64j+64] = [T_b.re | T_b.im], b = 4g+j
            psA = ps_a.tile([P, 4 * 2 * F], F32, tag="psA")
            for j in range(4):
                b = 4 * g + j
                wr = sb_wp_re[:, b * P:(b + 1) * P]
                wi = sb_wp_im[:, b * P:(b + 1) * P]
                osl = np.s_[:, 2 * F * j:2 * F * (j + 1)]
                nc.tensor.matmul(psA[osl], wr, sb_cat1, start=True, stop=False)
                nc.tensor.matmul(psA[osl], wi, sb_cat2, start=False, stop=True)
            # gather re / im blocks (strided views) into fp16 tiles
            psA_v = psA.rearrange("p (j c) -> p j c", j=4)
            sA_re = rot.tile([P, P], F16, tag="sA_re")
            nc.vector.tensor_copy(sA_re.rearrange("p (j c) -> p j c", j=4),
                                  psA_v[:, :, 0:F])
            sA_im = rot.tile([P, P], F16, tag="sA_im")
            nc.scalar.copy(sA_im.rearrange("p (j c) -> p j c", j=4),
                           psA_v[:, :, F:2 * F])
            # PE transposes (fp16)
            psT_re = ps_t.tile([P, P], F16, tag="psT_re")
            nc.tensor.transpose(psT_re, sA_re, sb_id16)
            psT_im = ps_t.tile([P, P], F16, tag="psT_im")
            nc.tensor.transpose(psT_im, sA_im, sb_id16)
            gt_re = rot.tile([P, P], F16, tag="gt_re")
            nc.vector.tensor_copy(gt_re, psT_re)
            gt_im = rot.tile([P, P], F16, tag="gt_im")
            nc.scalar.copy(gt_im, psT_im)

            # stage B: block-diag fp16 matmuls
            bdr = sb_bd_r[:, g * P:(g + 1) * P]
            bdi = sb_bd_i[:, g * P:(g + 1) * P]
            bdin = sb_bd_in[:, g * P:(g + 1) * P]
            psB_re = ps_b.tile([P, P], F32, tag="psB_re")
            psB_im = ps_b.tile([P, P], F32, tag="psB_im")
            nc.tensor.matmul(psB_re, bdr, gt_re, start=True, stop=False)
            nc.tensor.matmul(psB_im, bdr, gt_im, start=True, stop=False)
            nc.tensor.matmul(psB_re, bdin, gt_im, start=False, stop=True)
            nc.tensor.matmul(psB_im, bdi, gt_re, start=False, stop=True)

            sq_re = rot.tile([P, P], F32, tag="sq_re")
            nc.scalar.square(sq_re, psB_re)
            sq_im = rot.tile([P, P], F32, tag="sq_im")
            nc.scalar.square(sq_im, psB_im)
            pr = prob_pool.tile([P, P], F32, tag="pr")
            nc.vector.tensor_tensor(pr, sq_re, sq_im, op=ADD)
            nc.sync.dma_start(out=d_probs.ap()[g], in_=pr)

    nc.compile()
    return nc


# ----------------------------------------------------------------------------
# entry point
# ----------------------------------------------------------------------------

_PROGRAM_CACHE = {}

# test-harness knobs (grading path leaves these untouched)
TRACE = False
LAST_RESULT = None


def kernel(initial_state, ts, pauli_obs, indices, params_x, params_zz):
    ts = np.asarray(ts)
    pauli_obs = np.asarray(pauli_obs)
    indices = np.asarray(indices)
    T = ts.shape[0]
    shots = indices.shape[2]
    assert T == NCORES, f"expected {NCORES} timesteps, got {T}"

    shared, per_core, K, perm = prepare_host_data(
        initial_state, ts, pauli_obs, params_x, params_zz)

    key = (K, perm)
    if key not in _PROGRAM_CACHE:
        _PROGRAM_CACHE[key] = build_program(K, perm)
    nc = _PROGRAM_CACHE[key]

    in_maps = [{**shared, **pc} for pc in per_core]
    res = run_bass_kernel_spmd(nc, in_maps, core_ids=list(range(NCORES)),
                               trace=TRACE)
    global LAST_RESULT
    LAST_RESULT = res

    out = np.zeros((T, B, shots), np.float32)
    idx = indices.astype(np.int64)
    for t in range(T):
        tiles = res.results[t]["probs"]          # (NG, 128, 128)
        probs = tiles.reshape(NG, 4, F, P).reshape(B, DIM)  # b=(g,j); m=q'<<7|p'
        out[t] = np.take_along_axis(probs, idx[t], axis=1)
    return out
